# revision 7
# baseline (speedup 1.0000x reference)
"""ContraCLM token-level contrastive loss on 8 Trainium2 NeuronCores.

Data-parallel over the batch: core b handles sample b (B=8). Per core,
with S=1536, D=1024, T=0.05:

  f_v = l2norm(h_v) with masked token rows zeroed (mask folded into the
  rsqrt scale);  F = [f1; f2]  (2S x D, bf16, stored transposed as [D, 2S])

  sim = F F^T computed as 24 x 6 grid of [128, 512] PSUM strips (K=1024).
  exp(sim/T) row sums come free from the ScalarE activation free-dim
  accumulator. Diagonal-block strips (self-sim and positive-counterpart
  entries, which land on 128-block diagonals because 2S is a multiple of
  128 and partner offset is S) get the diagonal zeroed via affine_select
  before a DVE row-sum instead.

  Masked columns were zeroed in F, so each masked column contributes
  exp(0)=1 to a row sum: subtract K0 = 2S - 2n afterwards.
  pos_sim is computed exactly in fp32 as a row-wise dot product.
  per_tok = log(Ng + exp(pos_sim/T)) - pos_sim/T; masked mean over 2n
  tokens; AllReduce-mean across the 8 cores.
"""

import sys

for _p in ("/opt/trn_rl_repo", "/opt/pypackages"):
    if _p not in sys.path:
        sys.path.append(_p)

from contextlib import ExitStack

import numpy as np

import bass_rust

import concourse.bass as bass
import concourse.tile as tile
from concourse import mybir
from concourse.bass_utils import run_bass_kernel_spmd
from concourse.masks import make_identity
from concourse.vector_clock import ScopedClock

# The walrus build in this container encodes at most 2 sync waits per
# instruction (bass_rust's inst_waits_full agrees), but Tile's semaphore
# assignment can attach more. Hoist excess waits onto unfusable same-engine
# NoOps immediately before the instruction — the engine executes its queue
# in order, so semantics are preserved.
_MAX_WAITS = 1


def _split_excess_waits(nc, ordered):
    for bb_name, insts in ordered.items():
        out = []
        changed = False
        for inst in insts:
            si = getattr(inst, "sync_info", None)
            waits = list(si.on_wait) if si is not None else []
            if len(waits) > _MAX_WAITS:
                changed = True
                extra, keep = waits[:-_MAX_WAITS], waits[-_MAX_WAITS:]
                for i in range(0, len(extra), _MAX_WAITS):
                    out.append(mybir.InstNoOp(
                        name=nc.get_next_instruction_name(),
                        sync_info=mybir.SyncInfo(
                            on_wait=extra[i:i + _MAX_WAITS], on_update=[]),
                        bass_nofuse=True,
                        engine=inst.engine,
                    ))
                si.on_wait = keep
            out.append(inst)
        if changed:
            insts[:] = out


_orig_lower_ordered_insts = tile.TileContext._lower_ordered_insts


def _patched_lower_ordered_insts(self, ordered):
    _split_excess_waits(self.nc, ordered)
    return _orig_lower_ordered_insts(self, ordered)


tile.TileContext._lower_ordered_insts = _patched_lower_ordered_insts


def _split_waits_drain_and_barrier(self, tick_clock, wait_clock):
    nc = self.nc
    probe = nc.sync.nop(nofuse=True)
    wait_clock.add_sem_waits(
        probe.ins, ScopedClock({None: tick_clock.global_clock}))
    si = probe.ins.sync_info
    waits = list(si.on_wait) if si is not None else []
    if len(waits) > _MAX_WAITS:
        si.on_wait = waits[:_MAX_WAITS]
        for i in range(_MAX_WAITS, len(waits), _MAX_WAITS):
            nxt = nc.sync.nop(nofuse=True)
            nxt.ins.sync_info = bass_rust.SyncInfo(
                on_wait=waits[i:i + _MAX_WAITS], on_update=[])
    nc.sync.drain()
    nc.all_engine_barrier()
    assert self.sems is not None
    popped = nc._tile_sem_poison_stack.pop()
    assert popped is self._sem_poison
    nc.clear_and_free_semaphores(list(self.sems.allocated().values()))
    nc.all_engine_barrier()


tile.TileContext._drain_and_barrier = _split_waits_drain_and_barrier

S, D, NCORES = 1536, 1024, 8
ST = S // 128            # 12 s-tiles per view
NB = 2 * ST              # 24 block rows of F
NCS = 2 * S // 512       # 6 column strips
KT = D // 128            # 8 contraction tiles
TEMP_INV = 20.0          # 1 / 0.05
F32 = mybir.dt.float32
BF16 = mybir.dt.bfloat16
AF = mybir.ActivationFunctionType
ALU = mybir.AluOpType


def _build(num_devices: int = NCORES) -> bass.Bass:
    nc = bass.Bass(num_devices=num_devices)
    h1 = nc.dram_tensor("h1", [S, D], F32, kind="ExternalInput")
    h2 = nc.dram_tensor("h2", [S, D], F32, kind="ExternalInput")
    # mask, pre-laid-out host-side as [128, ST] so token t = 128*col + row
    maskT = nc.dram_tensor("maskT", [128, ST], F32, kind="ExternalInput")
    out = nc.dram_tensor("loss", [1, 1], F32, kind="ExternalOutput")

    with tile.TileContext(nc) as tc, ExitStack() as ctx:
        const_pool = ctx.enter_context(tc.tile_pool(name="const", bufs=1))
        big = ctx.enter_context(tc.tile_pool(name="big", bufs=1))
        stat = ctx.enter_context(tc.tile_pool(name="stat", bufs=1))

        ident = const_pool.tile([128, 128], BF16)
        make_identity(nc, ident[:])
        ones_col = const_pool.tile([128, 1], F32)
        nc.gpsimd.memset(ones_col[:], 1.0)
        ones_sq = const_pool.tile([128, 128], F32)
        nc.gpsimd.memset(ones_sq[:], 1.0)
        msk = const_pool.tile([128, ST], F32)
        nc.sync.dma_start(msk[:], maskT[:])

        fT = big.tile([128, KT, 2 * S], BF16)     # F^T, bf16, 6.3 MB
        s12 = stat.tile([128, ST], F32)           # raw <h1_i, h2_i>
        poss = stat.tile([128, ST], F32)          # pos_sim (masked rows -> 0)
        acc = stat.tile([128, NB, NCS], F32)      # per-strip row sums

        # ---- phase 1: load, norms, pos dot, normalize+cast, transpose ----
        with tc.tile_pool(name="ld", bufs=3) as ld, \
             tc.tile_pool(name="scr", bufs=2) as scr, \
             tc.tile_pool(name="sc", bufs=4) as scp, \
             tc.tile_pool(name="tp_ps", bufs=4, space="PSUM") as tps:
            for t in range(ST):
                ta = ld.tile([128, D], F32, tag="h")
                tb = ld.tile([128, D], F32, tag="h")
                nc.sync.dma_start(ta[:], h1[t * 128:(t + 1) * 128, :])
                nc.sync.dma_start(tb[:], h2[t * 128:(t + 1) * 128, :])

                sq1 = scr.tile([128, D], BF16, tag="sq")
                sq2 = scr.tile([128, D], BF16, tag="sq")
                ss1 = scp.tile([128, 1], F32, tag="ss")
                ss2 = scp.tile([128, 1], F32, tag="ss")
                nc.scalar.activation(sq1[:], ta[:], AF.Square, accum_out=ss1[:])
                nc.scalar.activation(sq2[:], tb[:], AF.Square, accum_out=ss2[:])

                nrm1 = scp.tile([128, 1], F32, tag="nrm")
                nrm2 = scp.tile([128, 1], F32, tag="nrm")
                nc.scalar.sqrt(nrm1[:], ss1[:])
                nc.scalar.sqrt(nrm2[:], ss2[:])
                ri1 = scp.tile([128, 1], F32, tag="ri")
                ri2 = scp.tile([128, 1], F32, tag="ri")
                nc.vector.reciprocal(ri1[:], nrm1[:])
                nc.vector.reciprocal(ri2[:], nrm2[:])
                sc1 = scp.tile([128, 1], F32, tag="msc")
                sc2 = scp.tile([128, 1], F32, tag="msc")
                nc.vector.tensor_mul(sc1[:], ri1[:], msk[:, t:t + 1])
                nc.vector.tensor_mul(sc2[:], ri2[:], msk[:, t:t + 1])

                # pos_sim = <h1, h2> * sc1 * sc2  (exact fp32; 0 if masked)
                prod = scr.tile([128, D], F32, tag="prod")
                nc.vector.tensor_mul(prod[:], ta[:], tb[:])
                nc.vector.tensor_reduce(s12[:, t:t + 1], prod[:],
                                        axis=mybir.AxisListType.X, op=ALU.add)
                ptmp = scp.tile([128, 1], F32, tag="ptmp")
                nc.vector.tensor_mul(ptmp[:], s12[:, t:t + 1], sc1[:])
                nc.vector.tensor_mul(poss[:, t:t + 1], ptmp[:], sc2[:])

                # normalize + cast to bf16, transpose into fT via PE
                for v, (src, scv) in enumerate(((ta, sc1), (tb, sc2))):
                    fn = scr.tile([128, D], BF16, tag="fn")
                    nc.vector.tensor_scalar_mul(fn[:], src[:], scv[:])
                    c0 = v * S + t * 128
                    for kg in range(2):
                        pt = tps.tile([128, 512], BF16)
                        for j in range(4):
                            k = kg * 4 + j
                            nc.tensor.transpose(
                                pt[:, j * 128:(j + 1) * 128],
                                fn[:, k * 128:(k + 1) * 128], ident[:])
                        nc.scalar.activation(
                            fT[:, kg * 4:(kg + 1) * 4, c0:c0 + 128],
                            pt[:].rearrange("p (j c) -> p j c", j=4),
                            AF.Copy)

        # ---- phase 2: sim strips, exp, row sums ----
        with tc.tile_pool(name="mm_ps", bufs=4, space="PSUM") as mmp, \
             tc.tile_pool(name="es", bufs=3) as esp:
            for cs in range(NCS):
                for r in range(NB):
                    ps = mmp.tile([128, 512], F32)
                    for k in range(KT):
                        nc.tensor.matmul(
                            ps[:],
                            fT[:, k, r * 128:(r + 1) * 128],
                            fT[:, k, cs * 512:(cs + 1) * 512],
                            start=(k == 0), stop=(k == KT - 1))
                    es = esp.tile([128, 512], BF16)
                    bad = [bc for bc in (r % ST, r % ST + ST)
                           if cs * 4 <= bc < cs * 4 + 4]
                    if bad:
                        jb = bad[0] - cs * 4
                        nc.scalar.activation(es[:], ps[:], AF.Exp,
                                             scale=TEMP_INV)
                        blk = es[:, jb * 128:(jb + 1) * 128]
                        nc.gpsimd.affine_select(
                            out=blk, in_=blk, compare_op=ALU.not_equal,
                            fill=0.0, base=0, pattern=[[-1, 128]],
                            channel_multiplier=1)
                        nc.vector.tensor_reduce(
                            acc[:, r, cs:cs + 1], es[:],
                            axis=mybir.AxisListType.X, op=ALU.add)
                    else:
                        nc.scalar.activation(es[:], ps[:], AF.Exp,
                                             scale=TEMP_INV,
                                             accum_out=acc[:, r, cs:cs + 1])

        # ---- phase 3: epilogue ----
        with tc.tile_pool(name="ep", bufs=1) as ep, \
             tc.tile_pool(name="ep_ps", bufs=2, space="PSUM") as epp:
            ng = ep.tile([128, NB], F32)
            nc.vector.tensor_reduce(ng[:], acc[:], axis=mybir.AxisListType.X,
                                    op=ALU.add)
            # n (valid token count) broadcast to every partition via ones matmul
            msum = ep.tile([128, 1], F32)
            nc.vector.tensor_reduce(msum[:], msk[:],
                                    axis=mybir.AxisListType.X, op=ALU.add)
            nps = epp.tile([128, 1], F32)
            nc.tensor.matmul(nps[:], ones_sq[:], msum[:], start=True,
                             stop=True)
            negK0 = ep.tile([128, 1], F32)  # 2n - 2S = -K0
            nc.scalar.activation(negK0[:], nps[:], AF.Copy, scale=2.0,
                                 bias=float(-2 * S))

            denom = ep.tile([128, NB], F32)
            nc.vector.tensor_scalar_add(denom[:], ng[:], negK0[:])
            pose = ep.tile([128, ST], F32)
            nc.scalar.activation(pose[:], poss[:], AF.Exp, scale=TEMP_INV)
            nc.vector.tensor_add(denom[:, 0:ST], denom[:, 0:ST], pose[:])
            nc.vector.tensor_add(denom[:, ST:NB], denom[:, ST:NB], pose[:])

            lg = ep.tile([128, NB], F32)
            nc.scalar.activation(lg[:], denom[:], AF.Ln)
            poss20 = ep.tile([128, ST], F32)
            nc.scalar.mul(poss20[:], poss[:], TEMP_INV)
            ptok = ep.tile([128, NB], F32)
            nc.vector.tensor_sub(ptok[:, 0:ST], lg[:, 0:ST], poss20[:])
            nc.vector.tensor_sub(ptok[:, ST:NB], lg[:, ST:NB], poss20[:])
            nc.vector.tensor_mul(ptok[:, 0:ST], ptok[:, 0:ST], msk[:])
            nc.vector.tensor_mul(ptok[:, ST:NB], ptok[:, ST:NB], msk[:])

            tsum = ep.tile([128, 1], F32)
            nc.vector.tensor_reduce(tsum[:], ptok[:],
                                    axis=mybir.AxisListType.X, op=ALU.add)
            lps = epp.tile([1, 1], F32)
            nc.tensor.matmul(lps[:], ones_col[:], tsum[:], start=True,
                             stop=True)
            # local contribution = total / (2n) / NCORES
            n16 = ep.tile([1, 1], F32)
            nc.scalar.activation(n16[:], nps[0:1, :], AF.Copy,
                                 scale=float(2 * num_devices))
            recn = ep.tile([1, 1], F32)
            nc.vector.reciprocal(recn[:], n16[:])
            lsb = ep.tile([1, 1], F32)
            nc.vector.tensor_mul(lsb[:], lps[:], recn[:])

            with tc.tile_pool(name="dram", bufs=1, space="DRAM") as dram:
                if num_devices > 1:
                    lin = dram.tile([1, 1], F32)
                    lout = dram.tile([1, 1], F32)
                    nc.sync.dma_start(lin[:], lsb[:])
                    nc.gpsimd.collective_compute(
                        "AllReduce", ALU.add,
                        replica_groups=[list(range(num_devices))],
                        ins=[lin.opt()], outs=[lout.opt()])
                    nc.sync.dma_start(out[:], lout[:])
                else:
                    nc.sync.dma_start(out[:], lsb[:])

    return nc


_NC = None


def _mask_layout(mask_row: np.ndarray) -> np.ndarray:
    # token t = 128 * col + row  ->  [128, ST]
    return np.ascontiguousarray(
        mask_row.astype(np.float32).reshape(ST, 128).T)


def kernel(last_hidden_states_1, last_hidden_states_2, token_mask_batch):
    global _NC
    h1 = np.ascontiguousarray(np.asarray(last_hidden_states_1,
                                         dtype=np.float32))
    h2 = np.ascontiguousarray(np.asarray(last_hidden_states_2,
                                         dtype=np.float32))
    mask = np.asarray(token_mask_batch)
    assert h1.shape == (NCORES, S, D), h1.shape

    if _NC is None:
        _NC = _build(NCORES)

    in_maps = [
        {"h1": h1[b], "h2": h2[b], "maskT": _mask_layout(mask[b])}
        for b in range(NCORES)
    ]
    res = run_bass_kernel_spmd(_NC, in_maps, list(range(NCORES)))
    loss = np.asarray(res.results[0]["loss"], dtype=np.float32).reshape(())
    return loss


# revision 16
# speedup vs baseline: 1.1926x; 1.1926x over previous
"""ContraCLM token-level contrastive loss on 8 Trainium2 NeuronCores.

Data-parallel over the batch: core b handles sample b (B=8). Per core,
with S=1536, D=1024, T=0.05:

  f_v = l2norm(h_v) with masked token rows zeroed (mask folded into the
  rsqrt scale);  F = [f1; f2]  (2S x D, bf16, stored transposed as [D, 2S])

  sim = F F^T computed as 24 x 6 grid of [128, 512] PSUM strips (K=1024).
  exp(sim/T) row sums come free from the ScalarE activation free-dim
  accumulator. Diagonal-block strips (self-sim and positive-counterpart
  entries, which land on 128-block diagonals because 2S is a multiple of
  128 and partner offset is S) get the diagonal zeroed via affine_select
  before a DVE row-sum instead.

  Masked columns were zeroed in F, so each masked column contributes
  exp(0)=1 to a row sum: subtract K0 = 2S - 2n afterwards.
  pos_sim is computed exactly in fp32 as a row-wise dot product.
  per_tok = log(Ng + exp(pos_sim/T)) - pos_sim/T; masked mean over 2n
  tokens; AllReduce-mean across the 8 cores.
"""

import sys

for _p in ("/opt/trn_rl_repo", "/opt/pypackages"):
    if _p not in sys.path:
        sys.path.append(_p)

from contextlib import ExitStack

import numpy as np

import bass_rust

import concourse.bass as bass
import concourse.tile as tile
from concourse import mybir
from concourse.bass_utils import run_bass_kernel_spmd
from concourse.masks import make_identity
from concourse.vector_clock import ScopedClock

# The walrus build in this container encodes at most 2 sync waits per
# instruction (bass_rust's inst_waits_full agrees), but Tile's semaphore
# assignment can attach more. Hoist excess waits onto unfusable same-engine
# NoOps immediately before the instruction — the engine executes its queue
# in order, so semantics are preserved.
_MAX_WAITS = 1


def _split_excess_waits(nc, ordered):
    for bb_name, insts in ordered.items():
        out = []
        changed = False
        for inst in insts:
            si = getattr(inst, "sync_info", None)
            waits = list(si.on_wait) if si is not None else []
            if len(waits) > _MAX_WAITS:
                changed = True
                extra, keep = waits[:-_MAX_WAITS], waits[-_MAX_WAITS:]
                for i in range(0, len(extra), _MAX_WAITS):
                    out.append(mybir.InstNoOp(
                        name=nc.get_next_instruction_name(),
                        sync_info=mybir.SyncInfo(
                            on_wait=extra[i:i + _MAX_WAITS], on_update=[]),
                        bass_nofuse=True,
                        engine=inst.engine,
                    ))
                si.on_wait = keep
            out.append(inst)
        if changed:
            insts[:] = out


_orig_lower_ordered_insts = tile.TileContext._lower_ordered_insts


def _patched_lower_ordered_insts(self, ordered):
    _split_excess_waits(self.nc, ordered)
    return _orig_lower_ordered_insts(self, ordered)


tile.TileContext._lower_ordered_insts = _patched_lower_ordered_insts


def _split_waits_drain_and_barrier(self, tick_clock, wait_clock):
    nc = self.nc
    probe = nc.sync.nop(nofuse=True)
    wait_clock.add_sem_waits(
        probe.ins, ScopedClock({None: tick_clock.global_clock}))
    si = probe.ins.sync_info
    waits = list(si.on_wait) if si is not None else []
    if len(waits) > _MAX_WAITS:
        si.on_wait = waits[:_MAX_WAITS]
        for i in range(_MAX_WAITS, len(waits), _MAX_WAITS):
            nxt = nc.sync.nop(nofuse=True)
            nxt.ins.sync_info = bass_rust.SyncInfo(
                on_wait=waits[i:i + _MAX_WAITS], on_update=[])
    nc.sync.drain()
    nc.all_engine_barrier()
    assert self.sems is not None
    popped = nc._tile_sem_poison_stack.pop()
    assert popped is self._sem_poison
    nc.clear_and_free_semaphores(list(self.sems.allocated().values()))
    nc.all_engine_barrier()


tile.TileContext._drain_and_barrier = _split_waits_drain_and_barrier

S, D, NCORES = 1536, 1024, 8
ST = S // 128            # 12 s-tiles per view
NB = 2 * ST              # 24 block rows of F
NCS = 2 * S // 512       # 6 column strips
KT = D // 128            # 8 contraction tiles
TEMP_INV = 20.0          # 1 / 0.05
F32 = mybir.dt.float32
BF16 = mybir.dt.bfloat16
AF = mybir.ActivationFunctionType
ALU = mybir.AluOpType


def _build(num_devices: int = NCORES, debug_dump: bool = False) -> bass.Bass:
    nc = bass.Bass(num_devices=num_devices)
    h1 = nc.dram_tensor("h1", [S, D], F32, kind="ExternalInput")
    h2 = nc.dram_tensor("h2", [S, D], F32, kind="ExternalInput")
    # mask, pre-laid-out host-side as [128, ST] so token t = 128*col + row
    maskT = nc.dram_tensor("maskT", [128, ST], F32, kind="ExternalInput")
    out = nc.dram_tensor("loss", [1, 1], F32, kind="ExternalOutput")
    if debug_dump:
        ng_dump = nc.dram_tensor("ng_dump", [128, NB], F32,
                                 kind="ExternalOutput")
        cacc_dump = nc.dram_tensor("cacc_dump", [128, ST], F32,
                                   kind="ExternalOutput")

    with tile.TileContext(nc) as tc, ExitStack() as ctx:
        const_pool = ctx.enter_context(tc.tile_pool(name="const", bufs=1))
        big = ctx.enter_context(tc.tile_pool(name="big", bufs=1))
        stat = ctx.enter_context(tc.tile_pool(name="stat", bufs=1))

        ident = const_pool.tile([128, 128], BF16)
        make_identity(nc, ident[:])
        ones_col = const_pool.tile([128, 1], F32)
        nc.gpsimd.memset(ones_col[:], 1.0)
        ones_sq = const_pool.tile([128, 128], F32)
        nc.gpsimd.memset(ones_sq[:], 1.0)
        ones_bf = const_pool.tile([128, 1], BF16)
        nc.gpsimd.memset(ones_bf[:], 1.0)
        msk = const_pool.tile([128, ST], F32)
        nc.sync.dma_start(msk[:], maskT[:])

        fT = big.tile([128, KT, 2 * S], BF16)     # F^T, bf16, 6.3 MB
        s12 = stat.tile([128, ST], F32)           # raw <h1_i, h2_i>
        poss = stat.tile([128, ST], F32)          # pos_sim (masked rows -> 0)
        acc = stat.tile([128, NB, NCS], F32)      # per-strip row sums
        cacc = stat.tile([128, ST], F32)          # B column sums (view-2 Ng)

        # ---- phase 1: load, norms, pos dot, normalize+cast, transpose ----
        with tc.tile_pool(name="ld", bufs=3) as ld, \
             tc.tile_pool(name="scr", bufs=2) as scr, \
             tc.tile_pool(name="sc", bufs=4) as scp, \
             tc.tile_pool(name="tp_ps", bufs=4, space="PSUM") as tps:
            for t in range(ST):
                ta = ld.tile([128, D], F32, tag="h")
                tb = ld.tile([128, D], F32, tag="h")
                nc.sync.dma_start(ta[:], h1[t * 128:(t + 1) * 128, :])
                nc.sync.dma_start(tb[:], h2[t * 128:(t + 1) * 128, :])

                sq1 = scr.tile([128, D], BF16, tag="sq")
                sq2 = scr.tile([128, D], BF16, tag="sq")
                ss1 = scp.tile([128, 1], F32, tag="ss")
                ss2 = scp.tile([128, 1], F32, tag="ss")
                nc.scalar.activation(sq1[:], ta[:], AF.Square, accum_out=ss1[:])
                nc.scalar.activation(sq2[:], tb[:], AF.Square, accum_out=ss2[:])

                nrm1 = scp.tile([128, 1], F32, tag="nrm")
                nrm2 = scp.tile([128, 1], F32, tag="nrm")
                nc.scalar.sqrt(nrm1[:], ss1[:])
                nc.scalar.sqrt(nrm2[:], ss2[:])
                ri1 = scp.tile([128, 1], F32, tag="ri")
                ri2 = scp.tile([128, 1], F32, tag="ri")
                nc.vector.reciprocal(ri1[:], nrm1[:])
                nc.vector.reciprocal(ri2[:], nrm2[:])
                sc1 = scp.tile([128, 1], F32, tag="msc")
                sc2 = scp.tile([128, 1], F32, tag="msc")
                nc.vector.tensor_mul(sc1[:], ri1[:], msk[:, t:t + 1])
                nc.vector.tensor_mul(sc2[:], ri2[:], msk[:, t:t + 1])

                # pos_sim = <h1, h2> * sc1 * sc2  (exact fp32; 0 if masked)
                prod = scr.tile([128, D], F32, tag="prod")
                nc.vector.tensor_mul(prod[:], ta[:], tb[:])
                nc.vector.tensor_reduce(s12[:, t:t + 1], prod[:],
                                        axis=mybir.AxisListType.X, op=ALU.add)
                ptmp = scp.tile([128, 1], F32, tag="ptmp")
                nc.vector.tensor_mul(ptmp[:], s12[:, t:t + 1], sc1[:])
                nc.vector.tensor_mul(poss[:, t:t + 1], ptmp[:], sc2[:])

                # normalize + cast to bf16, transpose into fT via PE
                for v, (src, scv) in enumerate(((ta, sc1), (tb, sc2))):
                    fn = scr.tile([128, D], BF16, tag="fn")
                    nc.vector.tensor_scalar_mul(fn[:], src[:], scv[:])
                    c0 = v * S + t * 128
                    for kg in range(2):
                        pt = tps.tile([128, 512], BF16)
                        for j in range(4):
                            k = kg * 4 + j
                            nc.tensor.transpose(
                                pt[:, j * 128:(j + 1) * 128],
                                fn[:, k * 128:(k + 1) * 128], ident[:])
                        nc.scalar.activation(
                            fT[:, kg * 4:(kg + 1) * 4, c0:c0 + 128],
                            pt[:].rearrange("p (j c) -> p j c", j=4),
                            AF.Copy)

        # ---- phase 2: sim strips, exp, row sums ----
        # Quadrants: A = f1 f1^T (rows 0..11, cs 0..2), B = f1 f2^T (rows
        # 0..11, cs 3..5), C = f2 f2^T (rows 12..23, cs 3..5). B^T's row
        # sums (the view-2 tokens' B contribution) are B's column sums,
        # accumulated in PSUM via tiny lhsT-stationary matmuls against ones.
        with tc.tile_pool(name="mm_ps", bufs=3, space="PSUM") as mmp, \
             tc.tile_pool(name="cb_ps", bufs=1, space="PSUM") as cbp, \
             tc.tile_pool(name="es", bufs=3) as esp:
            for cs in range(NCS):
                is_b_cs = cs >= NCS // 2
                rows = list(range(NB)) if is_b_cs else list(range(ST))
                if is_b_cs:
                    # one PSUM bank per block-column: matmul start=True
                    # resets the whole bank, so accumulation groups must
                    # not share banks
                    pcb = []
                    for jb in range(4):
                        pcb_jb = cbp.tile([128, 1], F32, tag=f"cb{jb}",
                                          name=f"pcb_{cs}_{jb}")
                        pcb.append(pcb_jb)
                for r in rows:
                    ps = mmp.tile([128, 512], F32)
                    for k in range(KT):
                        nc.tensor.matmul(
                            ps[:],
                            fT[:, k, r * 128:(r + 1) * 128],
                            fT[:, k, cs * 512:(cs + 1) * 512],
                            start=(k == 0), stop=(k == KT - 1))
                    es = esp.tile([128, 512], BF16)
                    bad = [bc for bc in (r % ST, r % ST + ST)
                           if cs * 4 <= bc < cs * 4 + 4]
                    if bad:
                        jb = bad[0] - cs * 4
                        nc.scalar.activation(es[:], ps[:], AF.Exp,
                                             scale=TEMP_INV)
                        blk = es[:, jb * 128:(jb + 1) * 128]
                        nc.gpsimd.affine_select(
                            out=blk, in_=blk, compare_op=ALU.not_equal,
                            fill=0.0, base=0, pattern=[[-1, 128]],
                            channel_multiplier=1)
                        nc.vector.tensor_reduce(
                            acc[:, r, cs:cs + 1], es[:],
                            axis=mybir.AxisListType.X, op=ALU.add)
                    else:
                        nc.scalar.activation(es[:], ps[:], AF.Exp,
                                             scale=TEMP_INV,
                                             accum_out=acc[:, r, cs:cs + 1])
                    if is_b_cs and r < ST:
                        for jb in range(4):
                            nc.tensor.matmul(
                                pcb[jb][:],
                                es[:, jb * 128:(jb + 1) * 128],
                                ones_bf[:],
                                start=(r == 0), stop=(r == ST - 1),
                                skip_group_check=True)
                if is_b_cs:
                    c0 = (cs - NCS // 2) * 4
                    for jb in range(4):
                        nc.scalar.activation(cacc[:, c0 + jb:c0 + jb + 1],
                                             pcb[jb][:], AF.Copy)

        # ---- phase 3: epilogue ----
        with tc.tile_pool(name="ep", bufs=1) as ep, \
             tc.tile_pool(name="ep_ps", bufs=2, space="PSUM") as epp:
            ng = ep.tile([128, NB], F32)
            nc.vector.tensor_reduce(ng[:, 0:ST], acc[:, 0:ST, :],
                                    axis=mybir.AxisListType.X, op=ALU.add)
            nc.vector.tensor_reduce(ng[:, ST:NB],
                                    acc[:, ST:NB, NCS // 2:NCS],
                                    axis=mybir.AxisListType.X, op=ALU.add)
            nc.vector.tensor_add(ng[:, ST:NB], ng[:, ST:NB], cacc[:])
            if debug_dump:
                nc.sync.dma_start(ng_dump[:], ng[:])
                nc.sync.dma_start(cacc_dump[:], cacc[:])
            # n (valid token count) broadcast to every partition via ones matmul
            msum = ep.tile([128, 1], F32)
            nc.vector.tensor_reduce(msum[:], msk[:],
                                    axis=mybir.AxisListType.X, op=ALU.add)
            nps = epp.tile([128, 1], F32)
            nc.tensor.matmul(nps[:], ones_sq[:], msum[:], start=True,
                             stop=True)
            negK0 = ep.tile([128, 1], F32)  # 2n - 2S = -K0
            nc.scalar.activation(negK0[:], nps[:], AF.Copy, scale=2.0,
                                 bias=float(-2 * S))

            denom = ep.tile([128, NB], F32)
            nc.vector.tensor_scalar_add(denom[:], ng[:], negK0[:])
            pose = ep.tile([128, ST], F32)
            nc.scalar.activation(pose[:], poss[:], AF.Exp, scale=TEMP_INV)
            nc.vector.tensor_add(denom[:, 0:ST], denom[:, 0:ST], pose[:])
            nc.vector.tensor_add(denom[:, ST:NB], denom[:, ST:NB], pose[:])

            lg = ep.tile([128, NB], F32)
            nc.scalar.activation(lg[:], denom[:], AF.Ln)
            poss20 = ep.tile([128, ST], F32)
            nc.scalar.mul(poss20[:], poss[:], TEMP_INV)
            ptok = ep.tile([128, NB], F32)
            nc.vector.tensor_sub(ptok[:, 0:ST], lg[:, 0:ST], poss20[:])
            nc.vector.tensor_sub(ptok[:, ST:NB], lg[:, ST:NB], poss20[:])
            nc.vector.tensor_mul(ptok[:, 0:ST], ptok[:, 0:ST], msk[:])
            nc.vector.tensor_mul(ptok[:, ST:NB], ptok[:, ST:NB], msk[:])

            tsum = ep.tile([128, 1], F32)
            nc.vector.tensor_reduce(tsum[:], ptok[:],
                                    axis=mybir.AxisListType.X, op=ALU.add)
            lps = epp.tile([1, 1], F32)
            nc.tensor.matmul(lps[:], ones_col[:], tsum[:], start=True,
                             stop=True)
            # local contribution = total / (2n) / NCORES
            n16 = ep.tile([1, 1], F32)
            nc.scalar.activation(n16[:], nps[0:1, :], AF.Copy,
                                 scale=float(2 * num_devices))
            recn = ep.tile([1, 1], F32)
            nc.vector.reciprocal(recn[:], n16[:])
            lsb = ep.tile([1, 1], F32)
            nc.vector.tensor_mul(lsb[:], lps[:], recn[:])

            with tc.tile_pool(name="dram", bufs=1, space="DRAM") as dram:
                if num_devices > 1:
                    lin = dram.tile([1, 1], F32)
                    lout = dram.tile([1, 1], F32)
                    nc.sync.dma_start(lin[:], lsb[:])
                    nc.gpsimd.collective_compute(
                        "AllReduce", ALU.add,
                        replica_groups=[list(range(num_devices))],
                        ins=[lin.opt()], outs=[lout.opt()])
                    nc.sync.dma_start(out[:], lout[:])
                else:
                    nc.sync.dma_start(out[:], lsb[:])

    return nc


_NC = None


def _mask_layout(mask_row: np.ndarray) -> np.ndarray:
    # token t = 128 * col + row  ->  [128, ST]
    return np.ascontiguousarray(
        mask_row.astype(np.float32).reshape(ST, 128).T)


def kernel(last_hidden_states_1, last_hidden_states_2, token_mask_batch):
    global _NC
    h1 = np.ascontiguousarray(np.asarray(last_hidden_states_1,
                                         dtype=np.float32))
    h2 = np.ascontiguousarray(np.asarray(last_hidden_states_2,
                                         dtype=np.float32))
    mask = np.asarray(token_mask_batch)
    assert h1.shape == (NCORES, S, D), h1.shape

    if _NC is None:
        _NC = _build(NCORES)

    in_maps = [
        {"h1": h1[b], "h2": h2[b], "maskT": _mask_layout(mask[b])}
        for b in range(NCORES)
    ]
    res = run_bass_kernel_spmd(_NC, in_maps, list(range(NCORES)))
    loss = np.asarray(res.results[0]["loss"], dtype=np.float32).reshape(())
    return loss


# revision 22
# speedup vs baseline: 1.6031x; 1.3442x over previous
"""ContraCLM token-level contrastive loss on 8 Trainium2 NeuronCores.

Data-parallel over the batch: core b handles sample b (B=8). Per core,
with S=1536, D=1024, T=0.05:

  f_v = l2norm(h_v) with masked token rows zeroed (mask folded into the
  rsqrt scale);  F = [f1; f2]  (2S x D, bf16, stored transposed as [D, 2S])

  sim = F F^T computed as 24 x 6 grid of [128, 512] PSUM strips (K=1024).
  exp(sim/T) row sums come free from the ScalarE activation free-dim
  accumulator. Diagonal-block strips (self-sim and positive-counterpart
  entries, which land on 128-block diagonals because 2S is a multiple of
  128 and partner offset is S) get the diagonal zeroed via affine_select
  before a DVE row-sum instead.

  Masked columns were zeroed in F, so each masked column contributes
  exp(0)=1 to a row sum: subtract K0 = 2S - 2n afterwards.
  pos_sim is computed exactly in fp32 as a row-wise dot product.
  per_tok = log(Ng + exp(pos_sim/T)) - pos_sim/T; masked mean over 2n
  tokens; AllReduce-mean across the 8 cores.
"""

import sys

for _p in ("/opt/trn_rl_repo", "/opt/pypackages"):
    if _p not in sys.path:
        sys.path.append(_p)

from contextlib import ExitStack

import numpy as np

import bass_rust

import concourse.bass as bass
import concourse.tile as tile
from concourse import mybir
from concourse.bass_utils import run_bass_kernel_spmd
from concourse.masks import make_identity
from concourse.vector_clock import ScopedClock

# The walrus build in this container encodes at most 2 sync waits per
# instruction (bass_rust's inst_waits_full agrees), but Tile's semaphore
# assignment can attach more. Hoist excess waits onto unfusable same-engine
# NoOps immediately before the instruction — the engine executes its queue
# in order, so semantics are preserved.
_MAX_WAITS = 1


def _split_excess_waits(nc, ordered):
    for bb_name, insts in ordered.items():
        out = []
        changed = False
        for inst in insts:
            si = getattr(inst, "sync_info", None)
            waits = list(si.on_wait) if si is not None else []
            if len(waits) > _MAX_WAITS:
                changed = True
                extra, keep = waits[:-_MAX_WAITS], waits[-_MAX_WAITS:]
                for i in range(0, len(extra), _MAX_WAITS):
                    out.append(mybir.InstNoOp(
                        name=nc.get_next_instruction_name(),
                        sync_info=mybir.SyncInfo(
                            on_wait=extra[i:i + _MAX_WAITS], on_update=[]),
                        bass_nofuse=True,
                        engine=inst.engine,
                    ))
                si.on_wait = keep
            out.append(inst)
        if changed:
            insts[:] = out


_orig_lower_ordered_insts = tile.TileContext._lower_ordered_insts


def _patched_lower_ordered_insts(self, ordered):
    _split_excess_waits(self.nc, ordered)
    return _orig_lower_ordered_insts(self, ordered)


tile.TileContext._lower_ordered_insts = _patched_lower_ordered_insts


def _split_waits_drain_and_barrier(self, tick_clock, wait_clock):
    nc = self.nc
    probe = nc.sync.nop(nofuse=True)
    wait_clock.add_sem_waits(
        probe.ins, ScopedClock({None: tick_clock.global_clock}))
    si = probe.ins.sync_info
    waits = list(si.on_wait) if si is not None else []
    if len(waits) > _MAX_WAITS:
        si.on_wait = waits[:_MAX_WAITS]
        for i in range(_MAX_WAITS, len(waits), _MAX_WAITS):
            nxt = nc.sync.nop(nofuse=True)
            nxt.ins.sync_info = bass_rust.SyncInfo(
                on_wait=waits[i:i + _MAX_WAITS], on_update=[])
    nc.sync.drain()
    nc.all_engine_barrier()
    assert self.sems is not None
    popped = nc._tile_sem_poison_stack.pop()
    assert popped is self._sem_poison
    nc.clear_and_free_semaphores(list(self.sems.allocated().values()))
    nc.all_engine_barrier()


tile.TileContext._drain_and_barrier = _split_waits_drain_and_barrier

S, D, NCORES = 1536, 1024, 8
ST = S // 128            # 12 s-tiles per view
NB = 2 * ST              # 24 block rows of F
NCS = 2 * S // 512       # 6 column strips
KT = D // 128            # 8 contraction tiles
TEMP_INV = 20.0          # 1 / 0.05
FP8_SCALE = 8.0          # f entries ~N(0, 1/32); x8 keeps them in e4m3's
                         # normal range (|f|*8 <~ 2, well under 240)
F32 = mybir.dt.float32
BF16 = mybir.dt.bfloat16
FP8 = mybir.dt.float8e4
AF = mybir.ActivationFunctionType
ALU = mybir.AluOpType


def _build(num_devices: int = NCORES, debug_dump: bool = False) -> bass.Bass:
    nc = bass.Bass(num_devices=num_devices)
    h1 = nc.dram_tensor("h1", [S, D], F32, kind="ExternalInput")
    h2 = nc.dram_tensor("h2", [S, D], F32, kind="ExternalInput")
    # mask, pre-laid-out host-side as [128, ST] so token t = 128*col + row
    maskT = nc.dram_tensor("maskT", [128, ST], F32, kind="ExternalInput")
    out = nc.dram_tensor("loss", [1, 1], F32, kind="ExternalOutput")
    if debug_dump:
        ng_dump = nc.dram_tensor("ng_dump", [128, NB], F32,
                                 kind="ExternalOutput")
        cacc_dump = nc.dram_tensor("cacc_dump", [128, ST], F32,
                                   kind="ExternalOutput")

    with tile.TileContext(nc) as tc, ExitStack() as ctx:
        const_pool = ctx.enter_context(tc.tile_pool(name="const", bufs=1))
        big = ctx.enter_context(tc.tile_pool(name="big", bufs=1))
        stat = ctx.enter_context(tc.tile_pool(name="stat", bufs=1))

        ident = const_pool.tile([128, 128], BF16)
        make_identity(nc, ident[:])
        ones_col = const_pool.tile([128, 1], F32)
        nc.gpsimd.memset(ones_col[:], 1.0)
        ones_sq = const_pool.tile([128, 128], F32)
        nc.gpsimd.memset(ones_sq[:], 1.0)
        ones_bf = const_pool.tile([128, 1], BF16)
        nc.gpsimd.memset(ones_bf[:], 1.0)
        msk = const_pool.tile([128, ST], F32)
        nc.sync.dma_start(msk[:], maskT[:])

        fT = big.tile([128, KT, 2 * S], FP8)      # F^T * 8, fp8e4, 3.1 MB
        s12 = stat.tile([128, ST], F32)           # raw <h1_i, h2_i>
        poss = stat.tile([128, ST], F32)          # pos_sim (masked rows -> 0)
        acc = stat.tile([128, NB, NCS], F32)      # per-strip row sums
        cacc = stat.tile([128, ST], F32)          # B column sums (view-2 Ng)

        # ---- phase 1: load, norms, pos dot, normalize+cast, transpose ----
        with tc.tile_pool(name="ld", bufs=3) as ld, \
             tc.tile_pool(name="scr", bufs=2) as scr, \
             tc.tile_pool(name="sc", bufs=4) as scp, \
             tc.tile_pool(name="tp_ps", bufs=4, space="PSUM") as tps:
            for t in range(ST):
                ta = ld.tile([128, D], F32, tag="h")
                tb = ld.tile([128, D], F32, tag="h")
                nc.sync.dma_start(ta[:], h1[t * 128:(t + 1) * 128, :])
                nc.sync.dma_start(tb[:], h2[t * 128:(t + 1) * 128, :])

                sq1 = scr.tile([128, D], BF16, tag="sq")
                sq2 = scr.tile([128, D], BF16, tag="sq")
                ss1 = scp.tile([128, 1], F32, tag="ss")
                ss2 = scp.tile([128, 1], F32, tag="ss")
                nc.scalar.activation(sq1[:], ta[:], AF.Square, accum_out=ss1[:])
                nc.scalar.activation(sq2[:], tb[:], AF.Square, accum_out=ss2[:])

                nrm1 = scp.tile([128, 1], F32, tag="nrm")
                nrm2 = scp.tile([128, 1], F32, tag="nrm")
                nc.scalar.sqrt(nrm1[:], ss1[:])
                nc.scalar.sqrt(nrm2[:], ss2[:])
                ri1 = scp.tile([128, 1], F32, tag="ri")
                ri2 = scp.tile([128, 1], F32, tag="ri")
                nc.vector.reciprocal(ri1[:], nrm1[:])
                nc.vector.reciprocal(ri2[:], nrm2[:])
                sc1 = scp.tile([128, 1], F32, tag="msc")
                sc2 = scp.tile([128, 1], F32, tag="msc")
                nc.vector.tensor_mul(sc1[:], ri1[:], msk[:, t:t + 1])
                nc.vector.tensor_mul(sc2[:], ri2[:], msk[:, t:t + 1])

                # pos_sim = <h1, h2> * sc1 * sc2  (exact fp32; 0 if masked)
                prod = scr.tile([128, D], F32, tag="prod")
                nc.vector.tensor_mul(prod[:], ta[:], tb[:])
                nc.vector.tensor_reduce(s12[:, t:t + 1], prod[:],
                                        axis=mybir.AxisListType.X, op=ALU.add)
                ptmp = scp.tile([128, 1], F32, tag="ptmp")
                nc.vector.tensor_mul(ptmp[:], s12[:, t:t + 1], sc1[:])
                nc.vector.tensor_mul(poss[:, t:t + 1], ptmp[:], sc2[:])

                # normalize + cast to bf16, transpose into fT via PE
                for v, (src, scv) in enumerate(((ta, sc1), (tb, sc2))):
                    fn = scr.tile([128, D], BF16, tag="fn")
                    nc.vector.tensor_scalar_mul(fn[:], src[:], scv[:])
                    c0 = v * S + t * 128
                    for kg in range(2):
                        pt = tps.tile([128, 512], BF16)
                        for j in range(4):
                            k = kg * 4 + j
                            nc.tensor.transpose(
                                pt[:, j * 128:(j + 1) * 128],
                                fn[:, k * 128:(k + 1) * 128], ident[:])
                        nc.vector.tensor_scalar_mul(
                            fT[:, kg * 4:(kg + 1) * 4, c0:c0 + 128],
                            pt[:].rearrange("p (j c) -> p j c", j=4),
                            FP8_SCALE)

        # ---- phase 2: sim strips, exp, row sums ----
        # Quadrants: A = f1 f1^T (rows 0..11, cs 0..2), B = f1 f2^T (rows
        # 0..11, cs 3..5), C = f2 f2^T (rows 12..23, cs 3..5). B^T's row
        # sums (the view-2 tokens' B contribution) are B's column sums,
        # accumulated in PSUM via tiny lhsT-stationary matmuls against ones.
        with tc.tile_pool(name="mm_ps", bufs=3, space="PSUM") as mmp, \
             tc.tile_pool(name="cb_ps", bufs=1, space="PSUM") as cbp, \
             tc.tile_pool(name="es", bufs=3) as esp:
            for cs in range(NCS):
                is_b_cs = cs >= NCS // 2
                rows = list(range(NB)) if is_b_cs else list(range(ST))
                if is_b_cs:
                    # one PSUM bank per block-column: matmul start=True
                    # resets the whole bank, so accumulation groups must
                    # not share banks
                    pcb = []
                    for jb in range(4):
                        pcb_jb = cbp.tile([128, 1], F32, tag=f"cb{jb}",
                                          name=f"pcb_{cs}_{jb}")
                        pcb.append(pcb_jb)
                for r in rows:
                    ps = mmp.tile([128, 512], F32)
                    for g in range(KT // 2):
                        nc.tensor.matmul(
                            ps[:],
                            fT[:, 2 * g:2 * g + 2, r * 128:(r + 1) * 128],
                            fT[:, 2 * g:2 * g + 2, cs * 512:(cs + 1) * 512],
                            perf_mode=mybir.MatmulPerfMode.DoubleRow,
                            start=(g == 0), stop=(g == KT // 2 - 1))
                    es = esp.tile([128, 512], BF16)
                    exp_scale = TEMP_INV / (FP8_SCALE * FP8_SCALE)
                    bad = [bc for bc in (r % ST, r % ST + ST)
                           if cs * 4 <= bc < cs * 4 + 4]
                    if bad:
                        jb = bad[0] - cs * 4
                        nc.scalar.activation(es[:], ps[:], AF.Exp,
                                             scale=exp_scale)
                        blk = es[:, jb * 128:(jb + 1) * 128]
                        nc.gpsimd.affine_select(
                            out=blk, in_=blk, compare_op=ALU.not_equal,
                            fill=0.0, base=0, pattern=[[-1, 128]],
                            channel_multiplier=1)
                        nc.vector.tensor_reduce(
                            acc[:, r, cs:cs + 1], es[:],
                            axis=mybir.AxisListType.X, op=ALU.add)
                    else:
                        nc.scalar.activation(es[:], ps[:], AF.Exp,
                                             scale=exp_scale,
                                             accum_out=acc[:, r, cs:cs + 1])
                    if is_b_cs and r < ST:
                        for jb in range(4):
                            nc.tensor.matmul(
                                pcb[jb][:],
                                es[:, jb * 128:(jb + 1) * 128],
                                ones_bf[:],
                                start=(r == 0), stop=(r == ST - 1),
                                skip_group_check=True)
                if is_b_cs:
                    c0 = (cs - NCS // 2) * 4
                    for jb in range(4):
                        nc.vector.tensor_copy(cacc[:, c0 + jb:c0 + jb + 1],
                                              pcb[jb][:])

        # ---- phase 3: epilogue ----
        with tc.tile_pool(name="ep", bufs=1) as ep, \
             tc.tile_pool(name="ep_ps", bufs=2, space="PSUM") as epp:
            ng = ep.tile([128, NB], F32)
            nc.vector.tensor_reduce(ng[:, 0:ST], acc[:, 0:ST, :],
                                    axis=mybir.AxisListType.X, op=ALU.add)
            nc.vector.tensor_reduce(ng[:, ST:NB],
                                    acc[:, ST:NB, NCS // 2:NCS],
                                    axis=mybir.AxisListType.X, op=ALU.add)
            nc.vector.tensor_add(ng[:, ST:NB], ng[:, ST:NB], cacc[:])
            if debug_dump:
                nc.sync.dma_start(ng_dump[:], ng[:])
                nc.sync.dma_start(cacc_dump[:], cacc[:])
            # n (valid token count) broadcast to every partition via ones matmul
            msum = ep.tile([128, 1], F32)
            nc.vector.tensor_reduce(msum[:], msk[:],
                                    axis=mybir.AxisListType.X, op=ALU.add)
            nps = epp.tile([128, 1], F32)
            nc.tensor.matmul(nps[:], ones_sq[:], msum[:], start=True,
                             stop=True)
            negK0 = ep.tile([128, 1], F32)  # 2n - 2S = -K0
            nc.scalar.activation(negK0[:], nps[:], AF.Copy, scale=2.0,
                                 bias=float(-2 * S))

            denom = ep.tile([128, NB], F32)
            nc.vector.tensor_scalar_add(denom[:], ng[:], negK0[:])
            pose = ep.tile([128, ST], F32)
            nc.scalar.activation(pose[:], poss[:], AF.Exp, scale=TEMP_INV)
            nc.vector.tensor_add(denom[:, 0:ST], denom[:, 0:ST], pose[:])
            nc.vector.tensor_add(denom[:, ST:NB], denom[:, ST:NB], pose[:])

            lg = ep.tile([128, NB], F32)
            nc.scalar.activation(lg[:], denom[:], AF.Ln)
            poss20 = ep.tile([128, ST], F32)
            nc.scalar.mul(poss20[:], poss[:], TEMP_INV)
            ptok = ep.tile([128, NB], F32)
            nc.vector.tensor_sub(ptok[:, 0:ST], lg[:, 0:ST], poss20[:])
            nc.vector.tensor_sub(ptok[:, ST:NB], lg[:, ST:NB], poss20[:])
            nc.vector.tensor_mul(ptok[:, 0:ST], ptok[:, 0:ST], msk[:])
            nc.vector.tensor_mul(ptok[:, ST:NB], ptok[:, ST:NB], msk[:])

            tsum = ep.tile([128, 1], F32)
            nc.vector.tensor_reduce(tsum[:], ptok[:],
                                    axis=mybir.AxisListType.X, op=ALU.add)
            lps = epp.tile([1, 1], F32)
            nc.tensor.matmul(lps[:], ones_col[:], tsum[:], start=True,
                             stop=True)
            # local contribution = total / (2n) / NCORES
            n16 = ep.tile([1, 1], F32)
            nc.scalar.activation(n16[:], nps[0:1, :], AF.Copy,
                                 scale=float(2 * num_devices))
            recn = ep.tile([1, 1], F32)
            nc.vector.reciprocal(recn[:], n16[:])
            lsb = ep.tile([1, 1], F32)
            nc.vector.tensor_mul(lsb[:], lps[:], recn[:])

            with tc.tile_pool(name="dram", bufs=1, space="DRAM") as dram:
                if num_devices > 1:
                    lin = dram.tile([1, 1], F32)
                    lout = dram.tile([1, 1], F32)
                    nc.sync.dma_start(lin[:], lsb[:])
                    nc.gpsimd.collective_compute(
                        "AllReduce", ALU.add,
                        replica_groups=[list(range(num_devices))],
                        ins=[lin.opt()], outs=[lout.opt()])
                    nc.sync.dma_start(out[:], lout[:])
                else:
                    nc.sync.dma_start(out[:], lsb[:])

    return nc


_NC = None


def _mask_layout(mask_row: np.ndarray) -> np.ndarray:
    # token t = 128 * col + row  ->  [128, ST]
    return np.ascontiguousarray(
        mask_row.astype(np.float32).reshape(ST, 128).T)


def kernel(last_hidden_states_1, last_hidden_states_2, token_mask_batch):
    global _NC
    h1 = np.ascontiguousarray(np.asarray(last_hidden_states_1,
                                         dtype=np.float32))
    h2 = np.ascontiguousarray(np.asarray(last_hidden_states_2,
                                         dtype=np.float32))
    mask = np.asarray(token_mask_batch)
    assert h1.shape == (NCORES, S, D), h1.shape

    if _NC is None:
        _NC = _build(NCORES)

    in_maps = [
        {"h1": h1[b], "h2": h2[b], "maskT": _mask_layout(mask[b])}
        for b in range(NCORES)
    ]
    res = run_bass_kernel_spmd(_NC, in_maps, list(range(NCORES)))
    loss = np.asarray(res.results[0]["loss"], dtype=np.float32).reshape(())
    return loss
